# revision 31
# baseline (speedup 1.0000x reference)
"""CoverageLoss kernel for 8 Trainium2 NeuronCores — "ship-all" design.

Math: loss = size(ls) + size(la) + cov(ss, ls) + cov(sa, la)
  cov(S, L): d = cdist_l1(S, L); sm4 = 4 smallest per row; tail = sm4.mean(-1)
             far = top64(tail); loss = mean(sm4[far]**2)

Device strategy (4 latent shards x 2 sample shards = 8 cores):
  One-sided thermometer quantization: latents are snapped to a Q=8 Lloyd
  ladder (per-coordinate, Gaussian), samples stay EXACT.  For sample value s
  and quantized latent c_j, |s - c_j| is linear in the thermometer bits
  g_q = sign(l - t_q), so a single fp8 DoubleRow matmul
  M[s, l] = <u(s), g(l)> gives d_q(s, l) = A(s) + M[s, l] = sum_e |s_e - c(l_e)|
  exactly (up to fp8 rounding of u).  The device ships the entire quantized
  distance matrix back (uint8 with per-sample affine on the ACT engine /
  fp16 on DVE); the host does top-24 candidate selection, exact fp32 L1
  refinement, top-64 far selection, and the final scalar.  Size losses are
  host-side (trivially small).
"""

import numpy as np
import ml_dtypes
from contextlib import ExitStack

import concourse.bass as bass
import concourse.bacc as bacc
import concourse.mybir as mybir
import concourse.tile as tile
from concourse.bass_utils import run_bass_kernel_spmd

# ---- problem constants ----
NLAT, ES, EA = 8192, 64, 32
NSMP = 2048
TAIL, FAR = 4, 64
A_SHARDS, B_SHARDS = 4, 2
NL = NLAT // A_SHARDS              # 2048 latents per core
MS = NSMP // B_SHARDS              # 1024 samples per core
NTILES = MS // 128                 # 8 sample tiles

Q_S = 2                            # thermometer levels per state coordinate
Q_A = 4                            # thermometer levels per action coordinate
KSUB_S = 1                         # plain 128-deep fp8 chunks
KSUB_A = 1
KC_S = ES * Q_S // (128 * KSUB_S)  # 1 chunk (states)
KC_A = EA * Q_A // (128 * KSUB_A)  # 1 chunk (actions)
NCHUNK = NL // 512                 # 4 psum column chunks

NCAND = 128                        # host-side candidate count per sample
U8_SCALE = 3.0                     # uint8 ladder: out = 250 - 3*d
U8_BIAS0 = 250.0

F32 = mybir.dt.float32
F16 = mybir.dt.float16
FP8 = mybir.dt.float8e4
U8 = mybir.dt.uint8

# psum half-tile consumer assignment: alternate ACT/DVE per half, ACT
# takes one extra (ACT's copy is slightly cheaper).  Both emit
# uint8 = relu(P + bias) with PSUM P = -3*M (-3 folded into coefficients).
def _on_act(half_idx):
    return half_idx % 2 == 0 or half_idx == 31


# ---- quantizer (host) ----
def _gauss_quantizer(Q):
    """Thresholds at Gaussian quantiles; ladder = Lloyd centroids."""
    from scipy.stats import norm
    qs = (np.arange(Q) + 0.5) / Q
    t = norm.ppf(qs)
    edges = np.concatenate([[-np.inf], t, [np.inf]])
    a, b = edges[:-1], edges[1:]
    c = (norm.pdf(a) - norm.pdf(b)) / np.maximum(norm.cdf(b) - norm.cdf(a), 1e-12)
    # solve-map:  |s - c_j| = A + sum_q u_q G[j, q],  G[j,q] = +1 if q<j else -1
    G = np.where(np.arange(Q)[None, :] < np.arange(Q + 1)[:, None], 1.0, -1.0)
    M = np.concatenate([np.ones((Q + 1, 1)), G], 1)
    return t.astype(np.float64), c.astype(np.float64), np.linalg.inv(M)


_QZ = {q: _gauss_quantizer(q) for q in {Q_S, Q_A}}


def _encode_samples(S, Q):
    """[n, e] samples -> (A_sum [n], U [n, e, Q] fp32 coefficient tensor)."""
    t, c, minv = _QZ[Q]
    B = np.abs(S.astype(np.float64)[..., None] - c)         # [n, e, Q+1]
    X = B @ minv.T                                           # [n, e, Q+1]
    A = X[..., 0].sum(-1)                                    # [n]
    U = X[..., 1:]                                           # [n, e, Q]
    return A.astype(np.float32), U.astype(np.float32)


def _encode_latents(L, Q):
    """[N, e] latents -> g [N, e, Q] in {-1, +1}."""
    t, c, minv = _QZ[Q]
    return np.where(L[:, :, None] > t.astype(np.float32), 1.0, -1.0
                    ).astype(np.float32)


def _to_dr_layout(X, kc, ksub):
    """[rows, e, Q] -> [128, kc, ksub, rows] fp8 layout.

    flat contraction index f = e*Q + q maps to (kc, ksub, kpart):
    f = ((kc*ksub) + ks)*128 + kpart.
    """
    rows = X.shape[0]
    Xf = X.reshape(rows, -1).T                               # [e*Q, rows]
    Xf = Xf.reshape(kc, ksub, 128, rows).transpose(2, 0, 1, 3)
    return np.ascontiguousarray(Xf).astype(ml_dtypes.float8_e4m3)


# ---- device kernel ----
def _build_nc():
    nc = bacc.Bacc("TRN2", target_bir_lowering=False, debug=False,
                   num_devices=8)
    inp = {}
    for name, shape in [
        ("bl_s", [128, KC_S, KSUB_S, NL]), ("bs_s", [128, KC_S, KSUB_S, MS]),
        ("bl_a", [128, KC_A, KSUB_A, NL]), ("bs_a", [128, KC_A, KSUB_A, MS]),
    ]:
        inp[name] = nc.dram_tensor(name, shape, FP8, kind="ExternalInput").ap()
    for name in ("biasv_s", "biasv_a"):
        inp[name] = nc.dram_tensor(name, [128, NTILES], F32,
                                   kind="ExternalInput").ap()
    inp["warmT"] = nc.dram_tensor("warmT", [128, 512], FP8,
                                  kind="ExternalInput").ap()
    qd8 = nc.dram_tensor("qd8", [128, NTILES, 2, NL], U8,
                         kind="ExternalOutput").ap()

    with tile.TileContext(nc) as tc:
        with ExitStack() as ctx:
            big = ctx.enter_context(tc.tile_pool(name="bigin", bufs=1))
            psum = ctx.enter_context(tc.tile_pool(name="psum", bufs=4,
                                                  space="PSUM"))
            st8 = ctx.enter_context(tc.tile_pool(name="st8", bufs=6))

            # warm-up weights first: tiny DMA, PE dummies run during the
            # big input DMAs so HAM is at 8/8 when real matmuls start
            warmt = big.tile([128, 512], FP8, tag="warmT")
            nc.sync.dma_start(warmt[:], inp["warmT"][:, :])
            psw = psum.tile([128, NL // 2], F32, tag="ps")
            for _ in range(4):
                nc.tensor.matmul(
                    psw[:, 0:512], lhsT=warmt[:, 0:128], rhs=warmt[:, 0:512],
                    start=True, stop=True, skip_group_check=True)

            # biases (small, needed by every consumer op)
            bias = {}
            for name in ("biasv_s", "biasv_a"):
                t = big.tile([128, NTILES], F32, tag=name)
                nc.sync.dma_start(t[:], inp[name][:, :])
                bias[name] = t
            # negated biases for the DVE path: out = (P - (-bias)) max 0
            nbias = {}
            for name in ("biasv_s", "biasv_a"):
                t = big.tile([128, NTILES], F32, tag=f"n{name}")
                nc.vector.tensor_scalar(
                    out=t[:], in0=bias[name][:], scalar1=-1.0, scalar2=None,
                    op0=mybir.AluOpType.mult)
                nbias[name] = t
            # encoded inputs: spread across queue engines and column-chunk
            # the big tables so the first matmuls' data lands early
            enc = {}
            for name, eng, nch in (("bs_s", nc.sync, 2),
                                   ("bl_s", nc.scalar, 4),
                                   ("bs_a", nc.gpsimd, 2),
                                   ("bl_a", nc.gpsimd, 4)):
                t = big.tile(list(inp[name].shape), FP8, tag=name)
                w = inp[name].shape[-1] // nch
                for j in range(nch):
                    eng.dma_start(t[:, 0, :, j * w:(j + 1) * w],
                                  inp[name][:, 0, :, j * w:(j + 1) * w])
                enc[name] = t

            # ACT table pre-warm (Relu) while DMAs stream
            warm8 = st8.tile([128, NL], U8, tag="qt8")
            nc.scalar.activation(
                warm8[:, 0:8], bias["biasv_s"][:, 0:8],
                mybir.ActivationFunctionType.Relu, bias=0.0, scale=1.0)

            covs = {
                "s": (enc["bl_s"], enc["bs_s"], bias["biasv_s"],
                      nbias["biasv_s"]),
                "a": (enc["bl_a"], enc["bs_a"], bias["biasv_a"],
                      nbias["biasv_a"]),
            }
            half_idx = 0
            for m in range(NTILES):
                for ci, cov in enumerate(("s", "a")):
                    qt = st8.tile([128, NL], U8, tag="qt8")
                    bl, bs, bv, nbv = covs[cov]
                    lhsT = bs[:, 0, 0, m * 128:(m + 1) * 128]
                    for h in range(2):
                        ps = psum.tile([128, NL // 2], F32, tag="ps")
                        for n in range(2):
                            j = h * 2 + n
                            nc.tensor.matmul(
                                ps[:, n * 512:(n + 1) * 512],
                                lhsT=lhsT,
                                rhs=bl[:, 0, 0, j * 512:(j + 1) * 512],
                                start=True, stop=True,
                                skip_group_check=True)
                        qslice = qt[:, h * (NL // 2):(h + 1) * (NL // 2)]
                        if _on_act(half_idx):
                            nc.scalar.activation(
                                qslice, ps[:],
                                mybir.ActivationFunctionType.Relu,
                                bias=bv[:, m:m + 1], scale=1.0)
                        else:
                            nc.vector.tensor_scalar(
                                out=qslice, in0=ps[:],
                                scalar1=nbv[:, m:m + 1],
                                scalar2=0.0, op0=mybir.AluOpType.subtract,
                                op1=mybir.AluOpType.max)
                        half_idx += 1
                    eng = nc.sync if (m * 2 + ci) % 2 == 0 else nc.gpsimd
                    eng.dma_start(qd8[:, m, ci, :], qt[:])
    nc.compile()
    return nc


_NC_CACHE = {}


def _get_nc():
    if "nc" not in _NC_CACHE:
        _NC_CACHE["nc"] = _build_nc()
    return _NC_CACHE["nc"]


# ---- host pre/post ----
def _make_in_maps(latent_states, latent_actions, state_space_samples,
                  action_space_samples):
    g_s = _encode_latents(latent_states, Q_S)   # [8192, 64, Q_S]
    g_a = _encode_latents(latent_actions, Q_A)  # [8192, 32, Q_A]
    A_s, U_s = _encode_samples(state_space_samples, Q_S)
    A_a, U_a = _encode_samples(action_space_samples, Q_A)
    warm = np.full((128, 512), 0.25, ml_dtypes.float8_e4m3)

    in_maps = []
    host = []                                  # per-core host context
    for core in range(8):
        a, b = core % A_SHARDS, core // A_SHARDS
        sl_l = slice(a * NL, (a + 1) * NL)
        sl_m = slice(b * MS, (b + 1) * MS)
        A_sb = A_s[sl_m]
        A_ab = A_a[sl_m]
        bias_s = (U8_BIAS0 - U8_SCALE *
                  A_sb.reshape(NTILES, 128).T).astype(np.float32)
        bias_a = (U8_BIAS0 - U8_SCALE *
                  A_ab.reshape(NTILES, 128).T).astype(np.float32)
        # -U8_SCALE folded into the sample coefficients: PSUM P = -3*M
        in_maps.append({
            "bl_s": _to_dr_layout(g_s[sl_l], KC_S, KSUB_S),
            "bs_s": _to_dr_layout(-U8_SCALE * U_s[sl_m], KC_S, KSUB_S),
            "bl_a": _to_dr_layout(g_a[sl_l], KC_A, KSUB_A),
            "bs_a": _to_dr_layout(-U8_SCALE * U_a[sl_m], KC_A, KSUB_A),
            "biasv_s": np.ascontiguousarray(bias_s),
            "biasv_a": np.ascontiguousarray(bias_a),
            "warmT": warm,
        })
        host.append({"a": a, "b": b})
    return in_maps, host


def _cov_loss_host(results, host, cov, samples, latents):
    """Assemble quantized rankings, exact-refine top candidates, compute
    the coverage loss term."""
    ci = 0 if cov == "s" else 1
    sm4_all = np.empty((NSMP, TAIL), np.float32)
    for b in range(B_SHARDS):
        cores = [b * A_SHARDS + a for a in range(A_SHARDS)]
        # rank score: larger = closer (uint8 = relu(250 - 3*d))
        score = np.empty((MS, A_SHARDS * NL), np.uint8)
        for a, c in enumerate(cores):
            r8 = results[c]["qd8"]    # [128, NTILES, 2, NL] uint8
            score[:, a * NL:(a + 1) * NL] = \
                r8[:, :, ci, :].transpose(1, 0, 2).reshape(MS, NL)
        idx = np.argpartition(-score.astype(np.int16), NCAND,
                              axis=1)[:, :NCAND]
        smp = samples[b * MS:(b + 1) * MS]
        cand = latents[idx]                          # [MS, NCAND, e]
        d_ex = np.abs(smp[:, None, :] - cand).sum(-1, dtype=np.float32)
        d_ex.sort(axis=1)
        sm4_all[b * MS:(b + 1) * MS] = d_ex[:, :TAIL]
    tails = sm4_all.mean(-1)
    far = np.argsort(-tails)[:FAR]
    return float((sm4_all[far].astype(np.float64) ** 2).mean())


def _size_loss_host(latents):
    norms = np.abs(latents).sum(-1, dtype=np.float64)
    viol = np.maximum(norms - 1.0, 0.0)
    return float((viol ** 2).mean())


def kernel(latent_states, latent_actions, state_space_samples,
           action_space_samples, _want_results=False, _trace=False):
    latent_states = np.asarray(latent_states, np.float32)
    latent_actions = np.asarray(latent_actions, np.float32)
    state_space_samples = np.asarray(state_space_samples, np.float32)
    action_space_samples = np.asarray(action_space_samples, np.float32)

    nc = _get_nc()
    in_maps, host = _make_in_maps(latent_states, latent_actions,
                                  state_space_samples, action_space_samples)
    res = run_bass_kernel_spmd(nc, in_maps, core_ids=list(range(8)),
                               trace=_trace)
    total = np.float64(0)
    total += _size_loss_host(latent_states)
    total += _size_loss_host(latent_actions)
    total += _cov_loss_host(res.results, host, "s", state_space_samples,
                            latent_states)
    total += _cov_loss_host(res.results, host, "a", action_space_samples,
                            latent_actions)
    out = np.float32(total)
    if _want_results:
        return out, res
    return out
